# revision 62
# baseline (speedup 1.0000x reference)
"""Trainium2 Bass kernel for the skip-LSTM autoencoder.

Strategy (8 NeuronCores, zero collectives):
  - Every core runs the full-batch (B=64) encoder + decoder recurrences
    replicated (they are latency-bound; replication costs nothing extra).
  - The 16384x16384 dense layer is column-sharded: core c computes output
    columns [2048c, 2048(c+1)).
  - Dense matmuls are W-stationary: stationary = 128x128 weight chunk,
    moving = h_t [128,64].  Cost-wise the PE charges moving rows only, so
    this halves dense PE time vs streaming W as the moving tensor, and the
    output lands as Y^T [2048, 64] accumulated in 2 PSUM banks across all
    128 timesteps, evacuated once with a fused (1/256)*psum + bias ACT.
  - Weight compression: k-tiles alternate float8e3 (E3M4, even t) and
    float16 (odd t), both pre-scaled by 2^8 on the host so a single
    dequant scale works.  48 MiB instead of 128 MiB fp32 / 64 MiB bf16
    per core; streamed via 32 grouped DMAs (4 k-tiles each) issued
    upfront on the otherwise-idle sync engine, ring-buffered in SBUF.
  - All recurrence state is float16 (3 more mantissa bits than bf16)
    to leave error budget for the fp8 weight half.

Structure facts exploited from the reference recurrence:
  - The i/f/o gate chains are self-contained (gate_t depends only on
    gate_{t-1} and x_t); only the candidate path consumes c_{t-1}.
  - h_t is consumed by nothing except the +16-step skip connection and
    the dense layer, so h is computed in batched tail groups off the
    critical path; the candidate recurrent matmul is distributed over
    c = m1 + fb so the serial cycle is tanh -> mul -> matmul -> tanh.

Scheduling discipline (the critical cycle is ~880ns: tanh exec 238 +
write-ack 211 + DVE m1 94 + ack 88 + matmul 27 + PE ack 177 + hops):
  - Decoder tail activations (skip sigmoid, batched tanh(c)) are split
    into [128,128] halves and pinned one-per-step AFTER the critical
    tanh_t with nosync scheduler-order edges (sigma -> tanh -> blob):
    they hide in the sigma-loop's ~130ns/step slack, which recovers over
    two steps.  Unpinned, the Tile list scheduler greedily hoists future
    blobs in front of the critical tanh; full-size 398ns blobs overrun
    any placement.
  - The decoder DVE stream is one nosync chain (fb -> m1 -> c_add ->
    tail vec ops) so a tail blob can never sit in front of a critical m1.
  - Same-engine nosync edges cost nothing; sync edges (chain_iter_dep)
    would add the producer's write-ack (~200ns) per link -- never use
    them for same-engine ordering.
"""

import sys
from contextlib import ExitStack

sys.path.insert(0, "/opt/trn_rl_repo")

import numpy as np
import ml_dtypes

import concourse.bass as bass
import concourse.mybir as mybir
import concourse.tile as tile
from concourse import bacc
from concourse.bass import _add_dep_helper
from concourse.bass_utils import run_bass_kernel_spmd


def _after(a, b):
    """Scheduler-only same-engine ordering: `a` must schedule after `b`.
    sync=False emits no semaphore -- pure Tile-scheduler ordering edge."""
    if b is not None:
        _add_dep_helper(a.ins, b.ins, sync=False, reason="manual order")
    return a

F16 = mybir.dt.float16
F8E3 = mybir.dt.float8e3
F32 = mybir.dt.float32
AF = mybir.ActivationFunctionType
ALU = mybir.AluOpType

U = 128      # units
B = 64       # batch
T = 128      # sequence length
M = 128      # input feature dim
SKIP = 16
NCORES = 8
JSH = (T * M) // NCORES  # 2048 output columns per core
NCH = JSH // U           # 16 dense column chunks of 128
WSCALE = 256.0           # host pre-scale on dense W (power of 2)
GSZ = 4                  # k-tiles per weight DMA group

nf16 = np.float16
ne3 = ml_dtypes.float8_e3m4

# module-level cache: (key) -> nc
_GRAPH_CACHE = {}


def _dec_tail2_act(nc, tc, pools, t0, c_hist, lo=0, n=2, after=None):
    """tanh(c) batch for steps t0+lo .. t0+lo+n-1 ([128, n*64] blob).
    Pinned right after the step's critical tanh so it absorbs into the
    sigma-loop's slack and never delays the candidate chain."""
    tmpb = pools["tmpb"]
    W = n * B
    cs0 = (t0 + lo) % 32
    tc_b = tmpb.tile([U, W], F16, tag="tc_b", name="tc_b")
    i = nc.scalar.activation(
        tc_b, c_hist[:, cs0:cs0 + n, :].rearrange("p s b -> p (s b)"),
        AF.Tanh)
    _after(i, after)
    return tc_b, i


def _dec_tail2_vec(nc, tc, pools, s0, t0, skp, tc_b, h_hist, ifo_ring,
                   lo=0, n=2, after=None):
    """DVE half for steps t0+lo..t0+lo+n-1: h_cell and skip blend with
    the s0 factors folded into the two scalar_tensor_tensor ops.  Chained
    into the decoder DVE stream (after this step's c_add) so it runs in
    the slack of the candidate round trip."""
    tmpb = pools["tmpb"]
    ALU_ = ALU
    W = n * B
    ws0 = (t0 + lo + SKIP) % 32
    hc = tmpb.tile([U, W], F16, tag="hc_b", name="hc_b")
    i1 = nc.vector.scalar_tensor_tensor(
        hc.rearrange("p (s b) -> p s b", s=n),
        ifo_ring[:, t0 % 16 + lo:t0 % 16 + lo + n, 2 * B:3 * B], s0,
        tc_b.rearrange("p (s b) -> p s b", s=n), ALU_.mult, ALU_.mult)
    _after(i1, after)
    i2 = nc.vector.scalar_tensor_tensor(
        h_hist[:, ws0:ws0 + n, :].rearrange("p s b -> p (s b)"),
        skp[:, lo * B:(lo + n) * B], 1.0 - s0, hc, ALU_.mult, ALU_.add)
    return i2


def _lstm_phase(nc, tc, pools, cfg, *, is_enc, k_tiles, rk_tiles, k2_tile,
                b2_col, bias_ifo, gsel_ifo, bias_c, ones_row, s0, xc_tile,
                ident, x1t_tile, c_hist, h_hist, ifo_ring, enc_h_chain,
                dense_fn, x1t_fetch=None, x0_tile=None):
    """Emit one skip-LSTM unroll (128 steps).

    is_enc: encoder (per-step x matmuls from x1t_tile, h only for the
            t%16==15 chain) vs decoder (constant xc via identity matmul,
            h for every step via 8-step batched tails + per-step dense).

    The candidate-gate pre-activation lives in its own PSUM bank so the
    i/f/o sigmoid (ACT read) never serializes against MM_c (PE write) --
    PSUM PE-write/ACT-read exclusion is bank-granular.
    """
    psum_g = pools["psum_g"]
    psum_c = pools["psum_c"]
    tmp = pools["tmp"]
    tmpb = pools["tmpb"]
    gp_ps = pools["skip_ps"]

    m1_prev = None
    fb_prev = None
    pend = None
    sk_sps = sk_skp = None
    tail_tc0 = tail_tc1 = None
    for t in range(T):
        # ---- gate pre-activations: ps_ifo [128, 192], ps_c [128, 64]
        # PSUM gotcha: start=True resets has_written for the WHOLE bank, so
        # exactly one full-width start matmul per bank must come first.
        ps_ifo = psum_g.tile([U, 3 * B], F32, tag="gates_ps")
        ps_c = psum_c.tile([U, B], F32, tag="cand_ps")
        if is_enc:
            # full-width start: bias broadcast via gate-select matmul
            nc.tensor.matmul(ps_ifo, bias_ifo, gsel_ifo, start=True,
                             stop=False)
            nc.tensor.matmul(ps_c, bias_c, ones_row[:, :B], start=True,
                             stop=False)
            # x-term: 4 matmuls kernel_g^T @ x_t
            for g in range(3):
                nc.tensor.matmul(ps_ifo[:, g * B:(g + 1) * B], k_tiles[g],
                                 x1t_tile[:, t, :], start=False, stop=(t == 0),
                                 skip_group_check=True)
            nc.tensor.matmul(ps_c, k_tiles[3], x1t_tile[:, t, :],
                             start=False, stop=(t == 0), skip_group_check=True)
        elif t == 0 and x0_tile is not None:
            # step 0 computed directly from RE (encoder-style gate-select
            # bias + dec_kernel matmuls) so the first sigma is not gated on
            # the xc SBUF copy; the xc copy proceeds in parallel for t>=1
            nc.tensor.matmul(ps_ifo, bias_ifo, gsel_ifo, start=True,
                             stop=False)
            nc.tensor.matmul(ps_c, bias_c, ones_row[:, :B], start=True,
                             stop=False)
            for g in range(3):
                nc.tensor.matmul(ps_ifo[:, g * B:(g + 1) * B], k_tiles[g],
                                 x0_tile, start=False, stop=True,
                                 skip_group_check=True)
            nc.tensor.matmul(ps_c, k_tiles[3], x0_tile,
                             start=False, stop=True, skip_group_check=True)
        else:
            # constant x-term (incl. bias) via identity matmuls
            nc.tensor.matmul(ps_ifo, ident, xc_tile[:, 0:3 * B], start=True,
                             stop=False)
            nc.tensor.matmul(ps_c, ident, xc_tile[:, 3 * B:4 * B], start=True,
                             stop=False)
        if t > 0:
            prev = ifo_ring[:, (t - 1) % 16, :]
            for g in range(3):
                rk = rk_tiles[g] if g < 2 else rk_tiles[3]
                nc.tensor.matmul(ps_ifo[:, g * B:(g + 1) * B], rk,
                                 prev[:, g * B:(g + 1) * B],
                                 start=False, stop=True, skip_group_check=True)
            # candidate recurrent term, distributed over c = m1 + fb so the
            # critical chain is tanh -> m1 -> matmul -> tanh (the c-add and
            # fb matmul run off the critical cycle)
            if t == 1:
                nc.tensor.matmul(ps_c, rk_tiles[2], c_hist[:, 0, :],
                                 start=False, stop=True, skip_group_check=True)
            else:
                nc.tensor.matmul(ps_c, rk_tiles[2], fb_prev,
                                 start=False, stop=False,
                                 skip_group_check=True)
                nc.tensor.matmul(ps_c, rk_tiles[2], m1_prev,
                                 start=False, stop=True, skip_group_check=True)
        # skip-gate matmul emitted two steps before its sigmoid (inputs
        # are 14+ steps old) so the gap-scheduled sigmoid is never
        # input-gated by the matmul's PE write-ack
        if (not is_enc) and t % 4 == 1 and t < T - 2:
            hs0 = (t - 1) % 32
            sk_sps = gp_ps.tile([U, 8 * B], F32, tag="skip_ps")
            nc.tensor.matmul(sk_sps[:, 0:4 * B], k2_tile,
                             h_hist[:, hs0:hs0 + 4, :],
                             start=True, stop=True, skip_group_check=True)

        # dense matmuls for the lagged timestep: lag 9 keeps h(t-9) three
        # full steps stale so the in-order PE stream never head-of-line
        # blocks on a just-landing blend.  The PE window that also carries
        # the 107ns skip matmul (t%4==2 emission -> window into t%4==3)
        # would overflow, so 4 of its 16 chunks shift to the next window.
        if dense_fn is not None and t >= 8:
            if t % 4 == 2 and t >= 10:
                dense_fn(t - 8, 0, 12)
            else:
                dense_fn(t - 8)
            if t % 4 == 3 and t >= 11:
                dense_fn(t - 9, 12, 16)

        # ---- activations: one sigmoid over [i,f,o], one tanh for cand.
        # Decoder: each step's ACT gap (the ~370ns candidate round trip
        # tanh -> m1 -> matmul) carries at most one ~290ns tail half-blob,
        # pinned there by scheduler-ordering edges sigma -> blob -> tanh
        # so Tile can neither hoist a future blob in front of the critical
        # tanh nor stack two blobs in one gap.
        ifo = ifo_ring[:, t % 16, :]
        i_sg = nc.scalar.activation(ifo, ps_ifo, AF.Sigmoid)
        u_t = tmp.tile([U, B], F16, tag="u")
        i_th = nc.scalar.activation(u_t, ps_c, AF.Tanh)
        i_blob = None
        if not is_enc:
            # tail blobs pinned AFTER the critical tanh: they hide in the
            # sigma-loop's ~130ns/step slack (which recovers over 2 steps)
            # instead of the c-loop's razor-thin sigma->tanh gap
            _after(i_th, i_sg)
            if t % 4 == 3:
                sk_skp = tmpb.tile([U, 4 * B], F16, tag="skp_b")
                i_blob = nc.scalar.activation(sk_skp[:, 0:2 * B],
                                              sk_sps[:, 0:2 * B],
                                              AF.Sigmoid, bias=b2_col)
                pend = (t - 3, sk_skp)
            elif t % 4 == 0 and t > 0:
                i_blob = nc.scalar.activation(sk_skp[:, 2 * B:4 * B],
                                              sk_sps[:, 2 * B:4 * B],
                                              AF.Sigmoid, bias=b2_col)
            elif t % 4 == 1 and t > 4:
                tail_tc0, i_blob = _dec_tail2_act(nc, tc, pools, t - 5,
                                                  c_hist, lo=0, n=2)
            elif t % 4 == 2 and t > 4:
                tail_tc1, i_blob = _dec_tail2_act(nc, tc, pools, t - 6,
                                                  c_hist, lo=2, n=2)
            if i_blob is not None:
                _after(i_blob, i_th)

        # ---- cell update: c_t = f*c_{t-1} + i*u   (f16 state)
        # decoder: the whole DVE stream (fb -> m1 -> c_add -> tail vec)
        # is nosync-chained so Tile cannot float a tail blob in front of
        # the critical m1; every chained op lands well before the next m1
        fb_eng = nc.gpsimd if is_enc else nc.vector
        if t == 0:
            i_c = nc.vector.tensor_tensor(c_hist[:, 0, :], ifo[:, 0:B], u_t,
                                          ALU.mult)
            if not is_enc:
                dve_prev = i_c
        else:
            m1 = tmp.tile([U, B], F16, tag="m1")
            fb = tmp.tile([U, B], F16, tag="fb")
            if is_enc:
                i_f = fb_eng.tensor_tensor(fb, ifo[:, B:2 * B],
                                           c_hist[:, (t - 1) % 32, :], ALU.mult)
                i_m = nc.vector.tensor_tensor(m1, ifo[:, 0:B], u_t, ALU.mult)
                i_c = nc.vector.tensor_tensor(c_hist[:, t % 32, :], m1, fb,
                                              ALU.add)
            else:
                i_f = fb_eng.tensor_tensor(fb, ifo[:, B:2 * B],
                                           c_hist[:, (t - 1) % 32, :],
                                           ALU.mult)
                i_m = nc.vector.tensor_tensor(m1, ifo[:, 0:B], u_t,
                                              ALU.mult)
                i_c = nc.vector.tensor_tensor(c_hist[:, t % 32, :], m1,
                                              fb, ALU.add)
                _after(i_f, dve_prev)
                _after(i_m, i_f)
                dve_prev = i_c
            m1_prev, fb_prev = m1, fb

        # ---- h tail
        if is_enc:
            # split like the decoder tails: skip-sigmoid at t%16==15, the
            # tanh/blend one step later (h_{t} isn't needed for 16 more
            # steps) so the ACT burst never exceeds the chain cycle
            def _enc_tail2(pend2):
                ci_, skp_, ts_ = pend2
                tc_t = tmp.tile([U, B], F16, tag="tc")
                nc.scalar.activation(tc_t, c_hist[:, ts_ % 32, :], AF.Tanh)
                hc = tmp.tile([U, B], F16, tag="hc")
                nc.vector.tensor_tensor(hc,
                                        ifo_ring[:, ts_ % 16, 2 * B:3 * B],
                                        tc_t, ALU.mult)
                skp2 = tmp.tile([U, B], F16, tag="skp2")
                nc.gpsimd.tensor_scalar_mul(skp2, skp_, 1.0 - s0)
                nc.vector.scalar_tensor_tensor(
                    enc_h_chain[ci_ + 1], hc, s0, skp2, ALU.mult, ALU.add)

            if t % SKIP == SKIP - 1:
                ci = t // SKIP          # chain index 0..7
                h_prev = enc_h_chain[ci]  # holds h_{t-16} (ci==0: zeros)
                sps = gp_ps.tile([U, 8 * B], F32, tag="skip_ps")
                nc.tensor.matmul(sps[:, 0:B], k2_tile, h_prev,
                                 start=True, stop=True, skip_group_check=True)
                skp = tmp.tile([U, B], F16, tag="skp")
                nc.scalar.activation(skp, sps[:, 0:B], AF.Sigmoid, bias=b2_col)
                pend = (ci, skp, t)
            if t % SKIP == 0 and t > 0:
                _enc_tail2(pend)
            if t == T - 1:
                _enc_tail2(pend)  # final link: RE = enc_h_chain[8]
        else:
            # DVE half of the tail group whose tanh half ran in this
            # step's ACT gap, chained after this step's c_add
            if t % 4 == 1 and t > 4:
                dve_prev = _dec_tail2_vec(nc, tc, pools, s0, t - 5, pend[1],
                                          tail_tc0, h_hist, ifo_ring,
                                          lo=0, n=2, after=dve_prev)
            elif t % 4 == 2 and t > 4:
                dve_prev = _dec_tail2_vec(nc, tc, pools, s0, t - 6, pend[1],
                                          tail_tc1, h_hist, ifo_ring,
                                          lo=2, n=2, after=dve_prev)

    if is_enc:
        return pend

    # ---- decoder drain: finish skip group T-4 and tail group T-4.
    # Order: tail tanh for steps 124/125 first (c ready, skp h1 in-loop),
    # then the skip h2 sigmoid, then tail tanh for 126/127 (waits c127).
    t0 = T - 4
    tcA, iA = _dec_tail2_act(nc, tc, pools, t0, c_hist, lo=0, n=2,
                             after=i_th)
    _dec_tail2_vec(nc, tc, pools, s0, t0, sk_skp, tcA, h_hist, ifo_ring,
                   lo=0, n=2, after=dve_prev)
    i_b = nc.scalar.activation(sk_skp[:, 2 * B:4 * B], sk_sps[:, 2 * B:4 * B],
                               AF.Sigmoid, bias=b2_col)
    _after(i_b, iA)
    tcB, iB = _dec_tail2_act(nc, tc, pools, t0, c_hist, lo=2, n=2,
                             after=i_b)
    _dec_tail2_vec(nc, tc, pools, s0, t0, sk_skp, tcB, h_hist, ifo_ring,
                   lo=2, n=2)
    return pend


def build_graph(cfg):
    """Build the SPMD graph (identical on all cores)."""
    nc = bacc.Bacc("TRN2", target_bir_lowering=False, debug=False,
                   num_devices=NCORES)

    # ---------------- DRAM parameters ----------------
    d_x1t = nc.dram_tensor("x1t", [M, T, B], F16, kind="ExternalInput").ap()
    d_enc_k = nc.dram_tensor("enc_k", [M, 4 * U], F16, kind="ExternalInput").ap()
    d_enc_rk = nc.dram_tensor("enc_rk", [U, 4 * U], F16, kind="ExternalInput").ap()
    d_enc_k2 = nc.dram_tensor("enc_k2", [U, U], F16, kind="ExternalInput").ap()
    d_dec_k = nc.dram_tensor("dec_k", [U, 4 * U], F16, kind="ExternalInput").ap()
    d_dec_rk = nc.dram_tensor("dec_rk", [U, 4 * U], F16, kind="ExternalInput").ap()
    d_dec_k2 = nc.dram_tensor("dec_k2", [U, U], F16, kind="ExternalInput").ap()
    d_enc_b4 = nc.dram_tensor("enc_b4", [4, U], F16, kind="ExternalInput").ap()
    d_enc_bc = nc.dram_tensor("enc_bc", [1, U], F16, kind="ExternalInput").ap()
    d_dec_b4 = nc.dram_tensor("dec_b4", [4, U], F16, kind="ExternalInput").ap()
    d_dec_bc = nc.dram_tensor("dec_bc", [1, U], F16, kind="ExternalInput").ap()
    d_gsel = nc.dram_tensor("gsel", [4, 4 * B], F16, kind="ExternalInput").ap()
    d_dec_b = nc.dram_tensor("dec_b", [1, 4 * U], F16, kind="ExternalInput").ap()
    d_enc_b2 = nc.dram_tensor("enc_b2", [U, 1], F32, kind="ExternalInput").ap()
    d_dec_b2 = nc.dram_tensor("dec_b2", [U, 1], F32, kind="ExternalInput").ap()
    d_ident = nc.dram_tensor("ident", [U, U], F16, kind="ExternalInput").ap()
    d_ones = nc.dram_tensor("ones", [1, 512], F16, kind="ExternalInput").ap()
    # dense weights: even k-tiles f8e3, odd k-tiles f16, both x256,
    # laid out [partition, tile, j] so group DMAs are contiguous
    d_we = nc.dram_tensor("we", [U, (T // 2) * JSH], F8E3,
                          kind="ExternalInput").ap()
    d_wf = nc.dram_tensor("wf", [U, (T // 2) * JSH], F16,
                          kind="ExternalInput").ap()
    # bias row [1, NCH*U] f16, pre-scaled by WSCALE: accumulated into the
    # dense PSUM banks via K=1 matmuls during warmup
    d_db = nc.dram_tensor("db", [1, NCH * U], F16, kind="ExternalInput").ap()
    d_out = nc.dram_tensor("out", [U, NCH * B], F32, kind="ExternalOutput").ap()

    enc_s0 = cfg["enc_s0"]
    dec_s0 = cfg["dec_s0"]

    with tile.TileContext(nc) as tc, ExitStack() as ctx:
        consts = ctx.enter_context(tc.tile_pool(name="consts", bufs=1))
        hist = ctx.enter_context(tc.tile_pool(name="hist", bufs=1))
        tmp = ctx.enter_context(tc.tile_pool(name="tmp", bufs=8))
        tmpb = ctx.enter_context(tc.tile_pool(name="tmpb", bufs=2))
        wpe = ctx.enter_context(tc.tile_pool(name="wpe", bufs=cfg["we_bufs"]))
        wpf = ctx.enter_context(tc.tile_pool(name="wpf", bufs=cfg["wf_bufs"]))
        psum_g = ctx.enter_context(tc.tile_pool(name="psum_g", bufs=3, space="PSUM"))
        psum_c = ctx.enter_context(tc.tile_pool(name="psum_c", bufs=2, space="PSUM"))
        skip_ps = ctx.enter_context(tc.tile_pool(name="skip_ps", bufs=1, space="PSUM"))
        psum_d = ctx.enter_context(tc.tile_pool(name="psum_d", bufs=1, space="PSUM"))

        pools = {"psum_g": psum_g, "psum_c": psum_c, "tmp": tmp,
                 "tmpb": tmpb, "skip_ps": skip_ps}

        # ---------------- load constants ----------------
        # small consts first (the DMA engines serve transfers in request
        # order: anything behind the 1MB x1t halves or the W stream waits)
        x1t_fetch = None

        def ld(eng, dram, shape, nm, dt=F16):
            t_ = consts.tile(shape, dt, tag=nm, name=nm)
            eng.dma_start(t_, dram)
            return t_

        # encoder-critical consts first on each queue, then x1t (smallest
        # chunk first so step 0 isn't gated on a 2.9us transfer), then the
        # decoder-only consts; the W stream (sync, below) queues last.
        # NOTHING issues from nc.scalar: each dma_start costs 667ns on the
        # ACT sequencer, which would serialize ahead of the first sigmoids.
        # step-0-critical DMAs split across BOTH queues so their DGE
        # generations overlap: sync gets the tiny bias rows + first x1t
        # sliver, gpsimd gets enc_k; everything else follows.
        x1t = hist.tile([M, T, B], F16)
        nc.sync.dma_start(x1t[:, 0:2, :], d_x1t[:, 0:2, :])
        gsel = ld(nc.sync, d_gsel, [4, 4 * B], "gsel_t")
        enc_b4 = ld(nc.sync, d_enc_b4, [4, U], "enc_b4_t")
        enc_bc = ld(nc.sync, d_enc_bc, [1, U], "enc_bc_t")
        ones = ld(nc.sync, d_ones, [1, 512], "ones_t")
        enc_k = ld(nc.gpsimd, d_enc_k, [M, 4 * U], "enc_k_t")
        enc_rk = ld(nc.gpsimd, d_enc_rk, [U, 4 * U], "enc_rk_t")
        ident = ld(nc.sync, d_ident, [U, U], "ident_t")
        nc.sync.dma_start(x1t[:, 2:8, :], d_x1t[:, 2:8, :])
        enc_b2 = ld(nc.sync, d_enc_b2, [U, 1], "enc_b2_t", F32)
        nc.gpsimd.dma_start(x1t[:, 8:32, :], d_x1t[:, 8:32, :])
        enc_k2 = ld(nc.gpsimd, d_enc_k2, [U, U], "enc_k2_t")
        nc.gpsimd.dma_start(x1t[:, 32:80, :], d_x1t[:, 32:80, :])
        nc.sync.dma_start(x1t[:, 80:T, :], d_x1t[:, 80:T, :])

        dec_rk = ld(nc.sync, d_dec_rk, [U, 4 * U], "dec_rk_t")
        dec_b2 = ld(nc.sync, d_dec_b2, [U, 1], "dec_b2_t", F32)
        dec_k2 = ld(nc.sync, d_dec_k2, [U, U], "dec_k2_t")
        dec_k = ld(nc.sync, d_dec_k, [U, 4 * U], "dec_k_t")
        dec_b = ld(nc.sync, d_dec_b, [1, 4 * U], "dec_b_t")
        dec_b4 = ld(nc.sync, d_dec_b4, [4, U], "dec_b4_t")
        dec_bc = ld(nc.sync, d_dec_bc, [1, U], "dec_bc_t")
        db_sb = ld(nc.sync, d_db, [1, NCH * U], "db_t")

        # ---------------- dense weight streaming ----------------
        # 32 grouped DMAs (alternating e3/f16, consumption order), all
        # issued upfront on the sync engine: the first we_bufs+wf_bufs
        # proceed during the encoder, the rest block SP in order until the
        # decoder's matmuls free ring slots.  SP does nothing else.
        we_tiles = []
        wf_tiles = []
        NGRP = (T // 2) // GSZ  # 16 groups per dtype
        for g in range(NGRP):
            te = wpe.tile([U, GSZ * JSH], F8E3, tag="we")
            nc.sync.dma_start(te, d_we[:, g * GSZ * JSH:(g + 1) * GSZ * JSH])
            we_tiles.append(te)
            tf = wpf.tile([U, GSZ * JSH], F16, tag="wf")
            nc.sync.dma_start(tf, d_wf[:, g * GSZ * JSH:(g + 1) * GSZ * JSH])
            wf_tiles.append(tf)

        # gate column order in psum: [i | f | o | cand]; reference weight
        # column order is [i | f | cand | o].  Map: psum gate 0->ref 0,
        # 1->ref 1, 2->ref 3, 3->ref 2.
        REF = [0, 1, 3, 2]
        enc_k_g = [enc_k[:, REF[g] * U:(REF[g] + 1) * U] for g in range(4)]
        dec_k_g = [dec_k[:, REF[g] * U:(REF[g] + 1) * U] for g in range(4)]
        enc_rk_g = [enc_rk[:, gg * U:(gg + 1) * U] for gg in range(4)]
        dec_rk_g = [dec_rk[:, gg * U:(gg + 1) * U] for gg in range(4)]

        # ---------------- state buffers ----------------
        c_hist = hist.tile([U, 32, B], F16)      # cell state ring
        h_hist = hist.tile([U, 32, B], F16)      # decoder h ring:
        # h(t) at slot (t+16)%32; slots 0..15 init to zero = h(-16..-1)
        ifo_ring = hist.tile([U, 16, 3 * B], F16)
        nc.vector.memset(h_hist[:, 0:SKIP, :], 0.0)
        zrow = consts.tile([1, 512], F16)
        nc.vector.memset(zrow, 0.0)

        # dense accumulators: Y^T [2048, 64] as 16 chunks [128, 64],
        # packed 8 per PSUM bank.  Zero each bank with one full-width
        # start matmul (K=1, zrow x zrow: no DMA dependency, so this runs
        # immediately and doubles as PE warm-up during the x1t wait).
        dense_ps = [psum_d.tile([U, 8 * B], F32, tag=f"dps{i}",
                                name=f"dps{i}") for i in range(2)]
        for i in range(2):
            nc.tensor.matmul(dense_ps[i], zrow[:, 0:U], zrow,
                             start=True, stop=False)

        zeros_h = consts.tile([U, B], F16)
        nc.vector.memset(zeros_h, 0.0)
        # warmup activations on a memset tile (NO DMA dependency):
        # front-load BOTH ACT table loads (sigmoid and tanh live in
        # different sets, ~1.3us each) so the first real activations
        # aren't gated on them
        warm = consts.tile([U, 1], F16)
        nc.scalar.activation(warm, zeros_h[:, 0:1], AF.Sigmoid)
        nc.scalar.activation(warm, zeros_h[:, 0:1], AF.Tanh)
        enc_h_chain = [zeros_h] + [
            hist.tile([U, B], F16, tag=f"ehc{i}", name=f"ehc{i}")
            for i in range(8)]

        def dense_fn(t_, c_lo=0, c_hi=NCH):
            # dense matmuls for one timestep: W-chunk stationary, h moving
            ti = t_ // 2
            if t_ % 2 == 0:
                wt = we_tiles[ti // GSZ]
            else:
                wt = wf_tiles[ti // GSZ]
            base = (ti % GSZ) * JSH
            lhs_h = h_hist[:, (t_ + SKIP) % 32, :]
            for ch in range(c_lo, c_hi):
                out = dense_ps[ch // 8][:, (ch % 8) * B:(ch % 8 + 1) * B]
                nc.tensor.matmul(out, wt[:, base + ch * U:base + (ch + 1) * U],
                                 lhs_h, start=False, stop=(t_ == T - 1),
                                 skip_group_check=True)

        # ---------------- encoder ----------------
        _lstm_phase(nc, tc, pools, cfg, is_enc=True,
                    k_tiles=enc_k_g, rk_tiles=enc_rk_g, k2_tile=enc_k2,
                    b2_col=enc_b2, bias_ifo=enc_b4[0:3, :],
                    gsel_ifo=gsel[0:3, 0:3 * B], bias_c=enc_bc,
                    ones_row=ones,
                    s0=enc_s0, xc_tile=None, ident=ident, x1t_tile=x1t,
                    c_hist=c_hist, h_hist=None, ifo_ring=ifo_ring,
                    enc_h_chain=enc_h_chain, dense_fn=None,
                    x1t_fetch=x1t_fetch)

        re_t = enc_h_chain[8]  # encoder h_127  [U, B] f16

        # fold dense bias into the accumulators between the phases (db has
        # long since landed; emitting this earlier would head-of-line block
        # the PE behind the db DMA during warmup)
        for ch in range(NCH):
            nc.tensor.matmul(dense_ps[ch // 8][:, (ch % 8) * B:(ch % 8 + 1) * B],
                             db_sb[:, ch * U:(ch + 1) * U], ones[:, :B],
                             start=False, stop=False, skip_group_check=True)

        # ---------------- decoder constant x-term ----------------
        # xc = dec_kernel^T @ RE + dec_bias, gate order [i|f|o|cand]
        xc_big = skip_ps.tile([U, 8 * B], F32, tag="skip_ps",
                              name="xc_big")
        xc_ps = xc_big[:, 0:4 * B]
        for g in range(4):
            gg = REF[g]
            gsl = xc_ps[:, g * B:(g + 1) * B]
            nc.tensor.matmul(gsl, dec_k[:, gg * U:(gg + 1) * U], re_t,
                             start=True, stop=False, skip_group_check=True)
            nc.tensor.matmul(gsl, dec_b[:, gg * U:(gg + 1) * U],
                             ones[:, :B], start=False, stop=True,
                             skip_group_check=True)
        xc = consts.tile([U, 4 * B], F16)
        nc.scalar.copy(xc, xc_ps)

        # ---------------- decoder (+ dense) ----------------
        dec_pend = _lstm_phase(nc, tc, pools, cfg, is_enc=False,
                    k_tiles=dec_k_g, rk_tiles=dec_rk_g, k2_tile=dec_k2,
                    b2_col=dec_b2, bias_ifo=dec_b4[0:3, :],
                    gsel_ifo=gsel[0:3, 0:3 * B], bias_c=dec_bc,
                    ones_row=ones,
                    s0=dec_s0, xc_tile=xc, ident=ident, x1t_tile=None,
                    c_hist=c_hist, h_hist=h_hist, ifo_ring=ifo_ring,
                    enc_h_chain=None, dense_fn=dense_fn, x0_tile=re_t)
        # dense epilogue (last few timesteps' k-tiles)
        for t_ in range(T - 8, T - 1):
            dense_fn(t_)

        # last k-tile: final matmuls, dequant and out-DMA pipelined in
        # [128,256] quarters (bank0 quarters on ACT, bank1 on DVE, DMAs
        # alternating between the sync HWDGE and Pool SWDGE queues)
        out_sb = consts.tile([U, NCH * B], F32)
        ti = (T - 1) // 2
        wt = wf_tiles[ti // GSZ]
        base = (ti % GSZ) * JSH
        lhs_h = h_hist[:, (T - 1 + SKIP) % 32, :]
        Q = 4 * B
        for q in range(4):
            for ch in range(4 * q, 4 * q + 4):
                out = dense_ps[ch // 8][:, (ch % 8) * B:(ch % 8 + 1) * B]
                nc.tensor.matmul(out, wt[:, base + ch * U:base + (ch + 1) * U],
                                 lhs_h, start=False, stop=True,
                                 skip_group_check=True)
            src = dense_ps[q // 2][:, (q % 2) * Q:(q % 2 + 1) * Q]
            dst = out_sb[:, q * Q:(q + 1) * Q]
            if q < 2:
                nc.scalar.activation(dst, src, AF.Identity, scale=1.0 / WSCALE)
            else:
                nc.vector.tensor_scalar_mul(dst, src, 1.0 / WSCALE)
            # four different DGE queues so descriptor generation for all
            # quarters runs in parallel; only the ~360ns transfers serialize
            eng = (nc.sync, nc.gpsimd, nc.scalar, nc.sync)[q]
            eng.dma_start(d_out[:, q * Q:(q + 1) * Q], dst)

    nc.compile()
    return nc


def _prep_inputs(X, enc_kernel, enc_rkernel, enc_kernel2, enc_bias, enc_bias2,
                 dec_kernel, dec_rkernel, dec_kernel2, dec_bias, dec_bias2,
                 dense_w, dense_b):
    """Host-side sharding/layout prep. Returns in_maps (list of 8 dicts)."""
    x1t = np.ascontiguousarray(
        X[:, 0].transpose(2, 1, 0)).astype(nf16)         # (M, T, B)
    common = {
        "x1t": x1t,
        "enc_k": enc_kernel.astype(nf16),
        "enc_rk": enc_rkernel.astype(nf16),
        "enc_k2": enc_kernel2.astype(nf16),
        "dec_k": dec_kernel.astype(nf16),
        "dec_rk": dec_rkernel.astype(nf16),
        "dec_k2": dec_kernel2.astype(nf16),
        # per-gate bias rows in device gate order [i|f|o|cand] for the K=4
        # gate-select start matmul
        "enc_b4": np.stack([enc_bias[r * U:(r + 1) * U]
                            for r in (0, 1, 3, 2)]).astype(nf16),
        "gsel": np.repeat(np.eye(4, dtype=np.float32), B, axis=1).astype(nf16),
        "enc_bc": enc_bias[2 * U:3 * U].reshape(1, -1).astype(nf16),
        "ones": np.ones((1, 512), np.float32).astype(nf16),
        "dec_b": dec_bias.reshape(1, -1).astype(nf16),
        "dec_b4": np.stack([dec_bias[r * U:(r + 1) * U]
                            for r in (0, 1, 3, 2)]).astype(nf16),
        "dec_bc": dec_bias[2 * U:3 * U].reshape(1, -1).astype(nf16),
        "enc_b2": enc_bias2.reshape(-1, 1).astype(np.float32),
        "dec_b2": dec_bias2.reshape(-1, 1).astype(np.float32),
        "ident": np.eye(U, dtype=np.float32).astype(nf16),
    }
    in_maps = []
    for c in range(NCORES):
        m = dict(common)
        # W^T shard [16384 k, 2048 j] -> [tile, p, j] -> [p, tile, j]
        wt = dense_w[c * JSH:(c + 1) * JSH, :].T.reshape(T, U, JSH)
        wt = np.ascontiguousarray(wt.transpose(1, 0, 2)) * WSCALE  # [p,t,j]
        m["we"] = np.ascontiguousarray(wt[:, 0::2, :]).reshape(U, -1).astype(ne3)
        m["wf"] = np.ascontiguousarray(wt[:, 1::2, :]).reshape(U, -1).astype(nf16)
        m["db"] = np.ascontiguousarray(
            dense_b[c * JSH:(c + 1) * JSH].reshape(1, -1) * WSCALE).astype(nf16)
        in_maps.append(m)
    return in_maps


def kernel(X, enc_kernel, enc_rkernel, enc_kernel2, enc_bias, enc_bias2,
           enc_s0, dec_kernel, dec_rkernel, dec_kernel2, dec_bias, dec_bias2,
           dec_s0, dense_w, dense_b, _trace=False):
    cfg = {
        "enc_s0": float(enc_s0),
        "dec_s0": float(dec_s0),
        "we_bufs": 6,
        "wf_bufs": 6,
    }
    key = tuple(sorted(cfg.items()))
    if key not in _GRAPH_CACHE:
        _GRAPH_CACHE[key] = build_graph(cfg)
    nc = _GRAPH_CACHE[key]

    in_maps = _prep_inputs(
        np.asarray(X), np.asarray(enc_kernel), np.asarray(enc_rkernel),
        np.asarray(enc_kernel2), np.asarray(enc_bias), np.asarray(enc_bias2),
        np.asarray(dec_kernel), np.asarray(dec_rkernel), np.asarray(dec_kernel2),
        np.asarray(dec_bias), np.asarray(dec_bias2),
        np.asarray(dense_w), np.asarray(dense_b))

    res = run_bass_kernel_spmd(nc, in_maps, core_ids=list(range(NCORES)),
                               trace=_trace)
    # out[c] is Y^T chunks: [p, ch*64+b] -> Y[b, c*2048 + ch*128 + p]
    parts = []
    for c in range(NCORES):
        buf = res.results[c]["out"].reshape(U, NCH, B)
        parts.append(buf.transpose(2, 1, 0).reshape(B, JSH))
    out = np.concatenate(parts, axis=1).reshape(B, T, M).astype(np.float32)
    if _trace:
        return out, res
    return out


if __name__ == "__main__":
    # smoke test with random data
    rng = np.random.default_rng(0)
    s_in = 1.0 / np.sqrt(M)
    s_u = 1.0 / np.sqrt(U)
    s_d = 1.0 / np.sqrt(T * M)
    inputs = {
        "X": rng.standard_normal((B, 2, T, M), dtype=np.float32),
        "enc_kernel": rng.standard_normal((M, 4 * U), dtype=np.float32) * s_in,
        "enc_rkernel": rng.standard_normal((U, 4 * U), dtype=np.float32) * s_u,
        "enc_kernel2": rng.standard_normal((U, U), dtype=np.float32) * s_u,
        "enc_bias": np.zeros(4 * U, np.float32),
        "enc_bias2": np.zeros(U, np.float32),
        "enc_s0": np.float32(0.5),
        "dec_kernel": rng.standard_normal((U, 4 * U), dtype=np.float32) * s_u,
        "dec_rkernel": rng.standard_normal((U, 4 * U), dtype=np.float32) * s_u,
        "dec_kernel2": rng.standard_normal((U, U), dtype=np.float32) * s_u,
        "dec_bias": np.zeros(4 * U, np.float32),
        "dec_bias2": np.zeros(U, np.float32),
        "dec_s0": np.float32(0.5),
        "dense_w": (rng.standard_normal((T * M, T * M), dtype=np.float32) * s_d),
        "dense_b": np.zeros(T * M, np.float32),
    }
    y = kernel(**inputs)
    print("kernel output", y.shape, y.dtype, float(np.abs(y).mean()))



# revision 63
# speedup vs baseline: 1.0024x; 1.0024x over previous
"""Trainium2 Bass kernel for the skip-LSTM autoencoder.

Strategy (8 NeuronCores, zero collectives):
  - Every core runs the full-batch (B=64) encoder + decoder recurrences
    replicated (they are latency-bound; replication costs nothing extra).
  - The 16384x16384 dense layer is column-sharded: core c computes output
    columns [2048c, 2048(c+1)).
  - Dense matmuls are W-stationary: stationary = 128x128 weight chunk,
    moving = h_t [128,64].  Cost-wise the PE charges moving rows only, so
    this halves dense PE time vs streaming W as the moving tensor, and the
    output lands as Y^T [2048, 64] accumulated in 2 PSUM banks across all
    128 timesteps, evacuated once with a fused (1/256)*psum + bias ACT.
  - Weight compression: k-tiles alternate float8e3 (E3M4, even t) and
    float16 (odd t), both pre-scaled by 2^8 on the host so a single
    dequant scale works.  48 MiB instead of 128 MiB fp32 / 64 MiB bf16
    per core; streamed via 32 grouped DMAs (4 k-tiles each) issued
    upfront on the otherwise-idle sync engine, ring-buffered in SBUF.
  - All recurrence state is float16 (3 more mantissa bits than bf16)
    to leave error budget for the fp8 weight half.

Structure facts exploited from the reference recurrence:
  - The i/f/o gate chains are self-contained (gate_t depends only on
    gate_{t-1} and x_t); only the candidate path consumes c_{t-1}.
  - h_t is consumed by nothing except the +16-step skip connection and
    the dense layer, so h is computed in batched tail groups off the
    critical path; the candidate recurrent matmul is distributed over
    c = m1 + fb so the serial cycle is tanh -> mul -> matmul -> tanh.

Scheduling discipline (the critical cycle is ~880ns: tanh exec 238 +
write-ack 211 + DVE m1 94 + ack 88 + matmul 27 + PE ack 177 + hops):
  - Decoder tail activations (skip sigmoid, batched tanh(c)) are split
    into [128,128] halves and pinned one-per-step AFTER the critical
    tanh_t with nosync scheduler-order edges (sigma -> tanh -> blob):
    they hide in the sigma-loop's ~130ns/step slack, which recovers over
    two steps.  Unpinned, the Tile list scheduler greedily hoists future
    blobs in front of the critical tanh; full-size 398ns blobs overrun
    any placement.
  - The decoder DVE stream is one nosync chain (fb -> m1 -> c_add ->
    tail vec ops) so a tail blob can never sit in front of a critical m1.
  - Same-engine nosync edges cost nothing; sync edges (chain_iter_dep)
    would add the producer's write-ack (~200ns) per link -- never use
    them for same-engine ordering.
"""

import sys
from contextlib import ExitStack

sys.path.insert(0, "/opt/trn_rl_repo")

import numpy as np
import ml_dtypes

import concourse.bass as bass
import concourse.mybir as mybir
import concourse.tile as tile
from concourse import bacc
from concourse.bass import _add_dep_helper
from concourse.bass_utils import run_bass_kernel_spmd


def _after(a, b):
    """Scheduler-only same-engine ordering: `a` must schedule after `b`.
    sync=False emits no semaphore -- pure Tile-scheduler ordering edge."""
    if b is not None:
        _add_dep_helper(a.ins, b.ins, sync=False, reason="manual order")
    return a

F16 = mybir.dt.float16
F8E3 = mybir.dt.float8e3
F32 = mybir.dt.float32
AF = mybir.ActivationFunctionType
ALU = mybir.AluOpType

U = 128      # units
B = 64       # batch
T = 128      # sequence length
M = 128      # input feature dim
SKIP = 16
NCORES = 8
JSH = (T * M) // NCORES  # 2048 output columns per core
NCH = JSH // U           # 16 dense column chunks of 128
WSCALE = 256.0           # host pre-scale on dense W (power of 2)
GSZ = 4                  # k-tiles per weight DMA group

nf16 = np.float16
ne3 = ml_dtypes.float8_e3m4

# module-level cache: (key) -> nc
_GRAPH_CACHE = {}


def _dec_tail2_act(nc, tc, pools, t0, c_hist, lo=0, n=2, after=None):
    """tanh(c) batch for steps t0+lo .. t0+lo+n-1 ([128, n*64] blob).
    Pinned right after the step's critical tanh so it absorbs into the
    sigma-loop's slack and never delays the candidate chain."""
    tmpb = pools["tmpb"]
    W = n * B
    cs0 = (t0 + lo) % 32
    tc_b = tmpb.tile([U, W], F16, tag="tc_b", name="tc_b")
    i = nc.scalar.activation(
        tc_b, c_hist[:, cs0:cs0 + n, :].rearrange("p s b -> p (s b)"),
        AF.Tanh)
    _after(i, after)
    return tc_b, i


def _dec_tail2_vec(nc, tc, pools, s0, t0, skp, tc_b, h_hist, ifo_ring,
                   lo=0, n=2, after=None):
    """DVE half for steps t0+lo..t0+lo+n-1: h_cell and skip blend with
    the s0 factors folded into the two scalar_tensor_tensor ops.  Chained
    into the decoder DVE stream (after this step's c_add) so it runs in
    the slack of the candidate round trip."""
    tmpb = pools["tmpb"]
    ALU_ = ALU
    W = n * B
    ws0 = (t0 + lo + SKIP) % 32
    hc = tmpb.tile([U, W], F16, tag="hc_b", name="hc_b")
    i1 = nc.vector.scalar_tensor_tensor(
        hc.rearrange("p (s b) -> p s b", s=n),
        ifo_ring[:, t0 % 16 + lo:t0 % 16 + lo + n, 2 * B:3 * B], s0,
        tc_b.rearrange("p (s b) -> p s b", s=n), ALU_.mult, ALU_.mult)
    _after(i1, after)
    i2 = nc.vector.scalar_tensor_tensor(
        h_hist[:, ws0:ws0 + n, :].rearrange("p s b -> p (s b)"),
        skp[:, lo * B:(lo + n) * B], 1.0 - s0, hc, ALU_.mult, ALU_.add)
    return i2


def _lstm_phase(nc, tc, pools, cfg, *, is_enc, k_tiles, rk_tiles, k2_tile,
                b2_col, bias_ifo, gsel_ifo, bias_c, ones_row, s0, xc_tile,
                ident, x1t_tile, c_hist, h_hist, ifo_ring, enc_h_chain,
                dense_fn, x1t_fetch=None, x0_tile=None):
    """Emit one skip-LSTM unroll (128 steps).

    is_enc: encoder (per-step x matmuls from x1t_tile, h only for the
            t%16==15 chain) vs decoder (constant xc via identity matmul,
            h for every step via 8-step batched tails + per-step dense).

    The candidate-gate pre-activation lives in its own PSUM bank so the
    i/f/o sigmoid (ACT read) never serializes against MM_c (PE write) --
    PSUM PE-write/ACT-read exclusion is bank-granular.
    """
    psum_g = pools["psum_g"]
    psum_c = pools["psum_c"]
    tmp = pools["tmp"]
    tmpb = pools["tmpb"]
    gp_ps = pools["skip_ps"]

    m1_prev = None
    fb_prev = None
    pend = None
    sk_sps = sk_skp = None
    tail_tc0 = tail_tc1 = None
    for t in range(T):
        # ---- gate pre-activations: ps_ifo [128, 192], ps_c [128, 64]
        # PSUM gotcha: start=True resets has_written for the WHOLE bank, so
        # exactly one full-width start matmul per bank must come first.
        ps_ifo = psum_g.tile([U, 3 * B], F32, tag="gates_ps")
        ps_c = psum_c.tile([U, B], F32, tag="cand_ps")
        if is_enc:
            # full-width start: bias broadcast via gate-select matmul
            nc.tensor.matmul(ps_ifo, bias_ifo, gsel_ifo, start=True,
                             stop=False)
            nc.tensor.matmul(ps_c, bias_c, ones_row[:, :B], start=True,
                             stop=False)
            # x-term: 4 matmuls kernel_g^T @ x_t
            for g in range(3):
                nc.tensor.matmul(ps_ifo[:, g * B:(g + 1) * B], k_tiles[g],
                                 x1t_tile[:, t, :], start=False, stop=(t == 0),
                                 skip_group_check=True)
            nc.tensor.matmul(ps_c, k_tiles[3], x1t_tile[:, t, :],
                             start=False, stop=(t == 0), skip_group_check=True)
        elif t == 0 and x0_tile is not None:
            # step 0 computed directly from RE (encoder-style gate-select
            # bias + dec_kernel matmuls) so the first sigma is not gated on
            # the xc SBUF copy; the xc copy proceeds in parallel for t>=1
            nc.tensor.matmul(ps_ifo, bias_ifo, gsel_ifo, start=True,
                             stop=False)
            nc.tensor.matmul(ps_c, bias_c, ones_row[:, :B], start=True,
                             stop=False)
            for g in range(3):
                nc.tensor.matmul(ps_ifo[:, g * B:(g + 1) * B], k_tiles[g],
                                 x0_tile, start=False, stop=True,
                                 skip_group_check=True)
            nc.tensor.matmul(ps_c, k_tiles[3], x0_tile,
                             start=False, stop=True, skip_group_check=True)
        else:
            # constant x-term (incl. bias) via identity matmuls
            nc.tensor.matmul(ps_ifo, ident, xc_tile[:, 0:3 * B], start=True,
                             stop=False)
            nc.tensor.matmul(ps_c, ident, xc_tile[:, 3 * B:4 * B], start=True,
                             stop=False)
        if t > 0:
            prev = ifo_ring[:, (t - 1) % 16, :]
            for g in range(3):
                rk = rk_tiles[g] if g < 2 else rk_tiles[3]
                nc.tensor.matmul(ps_ifo[:, g * B:(g + 1) * B], rk,
                                 prev[:, g * B:(g + 1) * B],
                                 start=False, stop=True, skip_group_check=True)
            # candidate recurrent term, distributed over c = m1 + fb so the
            # critical chain is tanh -> m1 -> matmul -> tanh (the c-add and
            # fb matmul run off the critical cycle)
            if t == 1:
                nc.tensor.matmul(ps_c, rk_tiles[2], c_hist[:, 0, :],
                                 start=False, stop=True, skip_group_check=True)
            else:
                nc.tensor.matmul(ps_c, rk_tiles[2], fb_prev,
                                 start=False, stop=False,
                                 skip_group_check=True)
                nc.tensor.matmul(ps_c, rk_tiles[2], m1_prev,
                                 start=False, stop=True, skip_group_check=True)
        # skip-gate matmul emitted two steps before its sigmoid (inputs
        # are 14+ steps old) so the gap-scheduled sigmoid is never
        # input-gated by the matmul's PE write-ack
        if (not is_enc) and t % 4 == 1 and t < T - 2:
            hs0 = (t - 1) % 32
            sk_sps = gp_ps.tile([U, 8 * B], F32, tag="skip_ps")
            nc.tensor.matmul(sk_sps[:, 0:4 * B], k2_tile,
                             h_hist[:, hs0:hs0 + 4, :],
                             start=True, stop=True, skip_group_check=True)

        # dense matmuls for the lagged timestep: lag 9 keeps h(t-9) three
        # full steps stale so the in-order PE stream never head-of-line
        # blocks on a just-landing blend.  The PE window that also carries
        # the 107ns skip matmul (t%4==2 emission -> window into t%4==3)
        # would overflow, so 4 of its 16 chunks shift to the next window.
        if dense_fn is not None and t >= 8:
            if t % 4 == 2 and t >= 10:
                dense_fn(t - 8, 0, 12)
            else:
                dense_fn(t - 8)
            if t % 4 == 3 and t >= 11:
                dense_fn(t - 9, 12, 16)

        # ---- activations: one sigmoid over [i,f,o], one tanh for cand.
        # Decoder: each step's ACT gap (the ~370ns candidate round trip
        # tanh -> m1 -> matmul) carries at most one ~290ns tail half-blob,
        # pinned there by scheduler-ordering edges sigma -> blob -> tanh
        # so Tile can neither hoist a future blob in front of the critical
        # tanh nor stack two blobs in one gap.
        ifo = ifo_ring[:, t % 16, :]
        i_sg = nc.scalar.activation(ifo, ps_ifo, AF.Sigmoid)
        u_t = tmp.tile([U, B], F16, tag="u")
        i_th = nc.scalar.activation(u_t, ps_c, AF.Tanh)
        i_blob = None
        if not is_enc:
            # tail blobs pinned AFTER the critical tanh: they hide in the
            # sigma-loop's ~130ns/step slack (which recovers over 2 steps)
            # instead of the c-loop's razor-thin sigma->tanh gap
            _after(i_th, i_sg)
            if t % 4 == 3:
                sk_skp = tmpb.tile([U, 4 * B], F16, tag="skp_b")
                i_blob = nc.scalar.activation(sk_skp[:, 0:2 * B],
                                              sk_sps[:, 0:2 * B],
                                              AF.Sigmoid, bias=b2_col)
                pend = (t - 3, sk_skp)
            elif t % 4 == 0 and t > 0:
                i_blob = nc.scalar.activation(sk_skp[:, 2 * B:4 * B],
                                              sk_sps[:, 2 * B:4 * B],
                                              AF.Sigmoid, bias=b2_col)
            elif t % 4 == 1 and t > 4:
                tail_tc0, i_blob = _dec_tail2_act(nc, tc, pools, t - 5,
                                                  c_hist, lo=0, n=2)
            elif t % 4 == 2 and t > 4:
                tail_tc1, i_blob = _dec_tail2_act(nc, tc, pools, t - 6,
                                                  c_hist, lo=2, n=2)
            if i_blob is not None:
                _after(i_blob, i_th)

        # ---- cell update: c_t = f*c_{t-1} + i*u   (f16 state)
        # decoder: the whole DVE stream (fb -> m1 -> c_add -> tail vec)
        # is nosync-chained so Tile cannot float a tail blob in front of
        # the critical m1; every chained op lands well before the next m1
        fb_eng = nc.gpsimd if is_enc else nc.vector
        if t == 0:
            i_c = nc.vector.tensor_tensor(c_hist[:, 0, :], ifo[:, 0:B], u_t,
                                          ALU.mult)
            if not is_enc:
                dve_prev = i_c
        else:
            m1 = tmp.tile([U, B], F16, tag="m1")
            fb = tmp.tile([U, B], F16, tag="fb")
            if is_enc:
                i_f = fb_eng.tensor_tensor(fb, ifo[:, B:2 * B],
                                           c_hist[:, (t - 1) % 32, :], ALU.mult)
                i_m = nc.vector.tensor_tensor(m1, ifo[:, 0:B], u_t, ALU.mult)
                i_c = nc.vector.tensor_tensor(c_hist[:, t % 32, :], m1, fb,
                                              ALU.add)
            else:
                i_f = fb_eng.tensor_tensor(fb, ifo[:, B:2 * B],
                                           c_hist[:, (t - 1) % 32, :],
                                           ALU.mult)
                i_m = nc.vector.tensor_tensor(m1, ifo[:, 0:B], u_t,
                                              ALU.mult)
                i_c = nc.vector.tensor_tensor(c_hist[:, t % 32, :], m1,
                                              fb, ALU.add)
                _after(i_f, dve_prev)
                _after(i_m, i_f)
                dve_prev = i_c
            m1_prev, fb_prev = m1, fb

        # ---- h tail
        if is_enc:
            # split like the decoder tails: skip-sigmoid at t%16==15, the
            # tanh/blend one step later (h_{t} isn't needed for 16 more
            # steps) so the ACT burst never exceeds the chain cycle
            def _enc_tail2(pend2):
                ci_, skp_, ts_ = pend2
                tc_t = tmp.tile([U, B], F16, tag="tc")
                _after(nc.scalar.activation(tc_t, c_hist[:, ts_ % 32, :],
                                            AF.Tanh), i_th)
                hc = tmp.tile([U, B], F16, tag="hc")
                nc.vector.tensor_tensor(hc,
                                        ifo_ring[:, ts_ % 16, 2 * B:3 * B],
                                        tc_t, ALU.mult)
                skp2 = tmp.tile([U, B], F16, tag="skp2")
                nc.gpsimd.tensor_scalar_mul(skp2, skp_, 1.0 - s0)
                nc.vector.scalar_tensor_tensor(
                    enc_h_chain[ci_ + 1], hc, s0, skp2, ALU.mult, ALU.add)

            if t % SKIP == SKIP - 1:
                ci = t // SKIP          # chain index 0..7
                h_prev = enc_h_chain[ci]  # holds h_{t-16} (ci==0: zeros)
                sps = gp_ps.tile([U, 8 * B], F32, tag="skip_ps")
                nc.tensor.matmul(sps[:, 0:B], k2_tile, h_prev,
                                 start=True, stop=True, skip_group_check=True)
                skp = tmp.tile([U, B], F16, tag="skp")
                _after(nc.scalar.activation(skp, sps[:, 0:B], AF.Sigmoid,
                                            bias=b2_col), i_th)
                pend = (ci, skp, t)
            if t % SKIP == 0 and t > 0:
                _enc_tail2(pend)
            if t == T - 1:
                _enc_tail2(pend)  # final link: RE = enc_h_chain[8]
        else:
            # DVE half of the tail group whose tanh half ran in this
            # step's ACT gap, chained after this step's c_add
            if t % 4 == 1 and t > 4:
                dve_prev = _dec_tail2_vec(nc, tc, pools, s0, t - 5, pend[1],
                                          tail_tc0, h_hist, ifo_ring,
                                          lo=0, n=2, after=dve_prev)
            elif t % 4 == 2 and t > 4:
                dve_prev = _dec_tail2_vec(nc, tc, pools, s0, t - 6, pend[1],
                                          tail_tc1, h_hist, ifo_ring,
                                          lo=2, n=2, after=dve_prev)

    if is_enc:
        return pend

    # ---- decoder drain: finish skip group T-4 and tail group T-4.
    # Order: tail tanh for steps 124/125 first (c ready, skp h1 in-loop),
    # then the skip h2 sigmoid, then tail tanh for 126/127 (waits c127).
    t0 = T - 4
    tcA, iA = _dec_tail2_act(nc, tc, pools, t0, c_hist, lo=0, n=2,
                             after=i_th)
    _dec_tail2_vec(nc, tc, pools, s0, t0, sk_skp, tcA, h_hist, ifo_ring,
                   lo=0, n=2, after=dve_prev)
    i_b = nc.scalar.activation(sk_skp[:, 2 * B:4 * B], sk_sps[:, 2 * B:4 * B],
                               AF.Sigmoid, bias=b2_col)
    _after(i_b, iA)
    tcB, iB = _dec_tail2_act(nc, tc, pools, t0, c_hist, lo=2, n=2,
                             after=i_b)
    _dec_tail2_vec(nc, tc, pools, s0, t0, sk_skp, tcB, h_hist, ifo_ring,
                   lo=2, n=2)
    return pend


def build_graph(cfg):
    """Build the SPMD graph (identical on all cores)."""
    nc = bacc.Bacc("TRN2", target_bir_lowering=False, debug=False,
                   num_devices=NCORES)

    # ---------------- DRAM parameters ----------------
    d_x1t = nc.dram_tensor("x1t", [M, T, B], F16, kind="ExternalInput").ap()
    d_enc_k = nc.dram_tensor("enc_k", [M, 4 * U], F16, kind="ExternalInput").ap()
    d_enc_rk = nc.dram_tensor("enc_rk", [U, 4 * U], F16, kind="ExternalInput").ap()
    d_enc_k2 = nc.dram_tensor("enc_k2", [U, U], F16, kind="ExternalInput").ap()
    d_dec_k = nc.dram_tensor("dec_k", [U, 4 * U], F16, kind="ExternalInput").ap()
    d_dec_rk = nc.dram_tensor("dec_rk", [U, 4 * U], F16, kind="ExternalInput").ap()
    d_dec_k2 = nc.dram_tensor("dec_k2", [U, U], F16, kind="ExternalInput").ap()
    d_enc_b4 = nc.dram_tensor("enc_b4", [4, U], F16, kind="ExternalInput").ap()
    d_enc_bc = nc.dram_tensor("enc_bc", [1, U], F16, kind="ExternalInput").ap()
    d_dec_b4 = nc.dram_tensor("dec_b4", [4, U], F16, kind="ExternalInput").ap()
    d_dec_bc = nc.dram_tensor("dec_bc", [1, U], F16, kind="ExternalInput").ap()
    d_gsel = nc.dram_tensor("gsel", [4, 4 * B], F16, kind="ExternalInput").ap()
    d_dec_b = nc.dram_tensor("dec_b", [1, 4 * U], F16, kind="ExternalInput").ap()
    d_enc_b2 = nc.dram_tensor("enc_b2", [U, 1], F32, kind="ExternalInput").ap()
    d_dec_b2 = nc.dram_tensor("dec_b2", [U, 1], F32, kind="ExternalInput").ap()
    d_ident = nc.dram_tensor("ident", [U, U], F16, kind="ExternalInput").ap()
    d_ones = nc.dram_tensor("ones", [1, 512], F16, kind="ExternalInput").ap()
    # dense weights: even k-tiles f8e3, odd k-tiles f16, both x256,
    # laid out [partition, tile, j] so group DMAs are contiguous
    d_we = nc.dram_tensor("we", [U, (T // 2) * JSH], F8E3,
                          kind="ExternalInput").ap()
    d_wf = nc.dram_tensor("wf", [U, (T // 2) * JSH], F16,
                          kind="ExternalInput").ap()
    # bias row [1, NCH*U] f16, pre-scaled by WSCALE: accumulated into the
    # dense PSUM banks via K=1 matmuls during warmup
    d_db = nc.dram_tensor("db", [1, NCH * U], F16, kind="ExternalInput").ap()
    d_out = nc.dram_tensor("out", [U, NCH * B], F32, kind="ExternalOutput").ap()

    enc_s0 = cfg["enc_s0"]
    dec_s0 = cfg["dec_s0"]

    with tile.TileContext(nc) as tc, ExitStack() as ctx:
        consts = ctx.enter_context(tc.tile_pool(name="consts", bufs=1))
        hist = ctx.enter_context(tc.tile_pool(name="hist", bufs=1))
        tmp = ctx.enter_context(tc.tile_pool(name="tmp", bufs=8))
        tmpb = ctx.enter_context(tc.tile_pool(name="tmpb", bufs=2))
        wpe = ctx.enter_context(tc.tile_pool(name="wpe", bufs=cfg["we_bufs"]))
        wpf = ctx.enter_context(tc.tile_pool(name="wpf", bufs=cfg["wf_bufs"]))
        psum_g = ctx.enter_context(tc.tile_pool(name="psum_g", bufs=3, space="PSUM"))
        psum_c = ctx.enter_context(tc.tile_pool(name="psum_c", bufs=2, space="PSUM"))
        skip_ps = ctx.enter_context(tc.tile_pool(name="skip_ps", bufs=1, space="PSUM"))
        psum_d = ctx.enter_context(tc.tile_pool(name="psum_d", bufs=1, space="PSUM"))

        pools = {"psum_g": psum_g, "psum_c": psum_c, "tmp": tmp,
                 "tmpb": tmpb, "skip_ps": skip_ps}

        # ---------------- load constants ----------------
        # small consts first (the DMA engines serve transfers in request
        # order: anything behind the 1MB x1t halves or the W stream waits)
        x1t_fetch = None

        def ld(eng, dram, shape, nm, dt=F16):
            t_ = consts.tile(shape, dt, tag=nm, name=nm)
            eng.dma_start(t_, dram)
            return t_

        # encoder-critical consts first on each queue, then x1t (smallest
        # chunk first so step 0 isn't gated on a 2.9us transfer), then the
        # decoder-only consts; the W stream (sync, below) queues last.
        # NOTHING issues from nc.scalar: each dma_start costs 667ns on the
        # ACT sequencer, which would serialize ahead of the first sigmoids.
        # step-0-critical DMAs split across BOTH queues so their DGE
        # generations overlap: sync gets the tiny bias rows + first x1t
        # sliver, gpsimd gets enc_k; everything else follows.
        x1t = hist.tile([M, T, B], F16)
        nc.sync.dma_start(x1t[:, 0:2, :], d_x1t[:, 0:2, :])
        gsel = ld(nc.sync, d_gsel, [4, 4 * B], "gsel_t")
        enc_b4 = ld(nc.sync, d_enc_b4, [4, U], "enc_b4_t")
        enc_bc = ld(nc.sync, d_enc_bc, [1, U], "enc_bc_t")
        ones = ld(nc.sync, d_ones, [1, 512], "ones_t")
        enc_k = ld(nc.gpsimd, d_enc_k, [M, 4 * U], "enc_k_t")
        enc_rk = ld(nc.gpsimd, d_enc_rk, [U, 4 * U], "enc_rk_t")
        ident = ld(nc.sync, d_ident, [U, U], "ident_t")
        nc.sync.dma_start(x1t[:, 2:8, :], d_x1t[:, 2:8, :])
        enc_b2 = ld(nc.sync, d_enc_b2, [U, 1], "enc_b2_t", F32)
        nc.gpsimd.dma_start(x1t[:, 8:32, :], d_x1t[:, 8:32, :])
        enc_k2 = ld(nc.gpsimd, d_enc_k2, [U, U], "enc_k2_t")
        nc.gpsimd.dma_start(x1t[:, 32:80, :], d_x1t[:, 32:80, :])
        nc.sync.dma_start(x1t[:, 80:T, :], d_x1t[:, 80:T, :])

        dec_rk = ld(nc.sync, d_dec_rk, [U, 4 * U], "dec_rk_t")
        dec_b2 = ld(nc.sync, d_dec_b2, [U, 1], "dec_b2_t", F32)
        dec_k2 = ld(nc.sync, d_dec_k2, [U, U], "dec_k2_t")
        dec_k = ld(nc.sync, d_dec_k, [U, 4 * U], "dec_k_t")
        dec_b = ld(nc.sync, d_dec_b, [1, 4 * U], "dec_b_t")
        dec_b4 = ld(nc.sync, d_dec_b4, [4, U], "dec_b4_t")
        dec_bc = ld(nc.sync, d_dec_bc, [1, U], "dec_bc_t")
        db_sb = ld(nc.sync, d_db, [1, NCH * U], "db_t")

        # ---------------- dense weight streaming ----------------
        # 32 grouped DMAs (alternating e3/f16, consumption order), all
        # issued upfront on the sync engine: the first we_bufs+wf_bufs
        # proceed during the encoder, the rest block SP in order until the
        # decoder's matmuls free ring slots.  SP does nothing else.
        we_tiles = []
        wf_tiles = []
        NGRP = (T // 2) // GSZ  # 16 groups per dtype
        for g in range(NGRP):
            te = wpe.tile([U, GSZ * JSH], F8E3, tag="we")
            nc.sync.dma_start(te, d_we[:, g * GSZ * JSH:(g + 1) * GSZ * JSH])
            we_tiles.append(te)
            tf = wpf.tile([U, GSZ * JSH], F16, tag="wf")
            nc.sync.dma_start(tf, d_wf[:, g * GSZ * JSH:(g + 1) * GSZ * JSH])
            wf_tiles.append(tf)

        # gate column order in psum: [i | f | o | cand]; reference weight
        # column order is [i | f | cand | o].  Map: psum gate 0->ref 0,
        # 1->ref 1, 2->ref 3, 3->ref 2.
        REF = [0, 1, 3, 2]
        enc_k_g = [enc_k[:, REF[g] * U:(REF[g] + 1) * U] for g in range(4)]
        dec_k_g = [dec_k[:, REF[g] * U:(REF[g] + 1) * U] for g in range(4)]
        enc_rk_g = [enc_rk[:, gg * U:(gg + 1) * U] for gg in range(4)]
        dec_rk_g = [dec_rk[:, gg * U:(gg + 1) * U] for gg in range(4)]

        # ---------------- state buffers ----------------
        c_hist = hist.tile([U, 32, B], F16)      # cell state ring
        h_hist = hist.tile([U, 32, B], F16)      # decoder h ring:
        # h(t) at slot (t+16)%32; slots 0..15 init to zero = h(-16..-1)
        ifo_ring = hist.tile([U, 16, 3 * B], F16)
        nc.vector.memset(h_hist[:, 0:SKIP, :], 0.0)
        zrow = consts.tile([1, 512], F16)
        nc.vector.memset(zrow, 0.0)

        # dense accumulators: Y^T [2048, 64] as 16 chunks [128, 64],
        # packed 8 per PSUM bank.  Zero each bank with one full-width
        # start matmul (K=1, zrow x zrow: no DMA dependency, so this runs
        # immediately and doubles as PE warm-up during the x1t wait).
        dense_ps = [psum_d.tile([U, 8 * B], F32, tag=f"dps{i}",
                                name=f"dps{i}") for i in range(2)]
        for i in range(2):
            nc.tensor.matmul(dense_ps[i], zrow[:, 0:U], zrow,
                             start=True, stop=False)

        zeros_h = consts.tile([U, B], F16)
        nc.vector.memset(zeros_h, 0.0)
        # warmup activations on a memset tile (NO DMA dependency):
        # front-load BOTH ACT table loads (sigmoid and tanh live in
        # different sets, ~1.3us each) so the first real activations
        # aren't gated on them
        warm = consts.tile([U, 1], F16)
        nc.scalar.activation(warm, zeros_h[:, 0:1], AF.Sigmoid)
        nc.scalar.activation(warm, zeros_h[:, 0:1], AF.Tanh)
        enc_h_chain = [zeros_h] + [
            hist.tile([U, B], F16, tag=f"ehc{i}", name=f"ehc{i}")
            for i in range(8)]

        def dense_fn(t_, c_lo=0, c_hi=NCH):
            # dense matmuls for one timestep: W-chunk stationary, h moving
            ti = t_ // 2
            if t_ % 2 == 0:
                wt = we_tiles[ti // GSZ]
            else:
                wt = wf_tiles[ti // GSZ]
            base = (ti % GSZ) * JSH
            lhs_h = h_hist[:, (t_ + SKIP) % 32, :]
            for ch in range(c_lo, c_hi):
                out = dense_ps[ch // 8][:, (ch % 8) * B:(ch % 8 + 1) * B]
                nc.tensor.matmul(out, wt[:, base + ch * U:base + (ch + 1) * U],
                                 lhs_h, start=False, stop=(t_ == T - 1),
                                 skip_group_check=True)

        # ---------------- encoder ----------------
        _lstm_phase(nc, tc, pools, cfg, is_enc=True,
                    k_tiles=enc_k_g, rk_tiles=enc_rk_g, k2_tile=enc_k2,
                    b2_col=enc_b2, bias_ifo=enc_b4[0:3, :],
                    gsel_ifo=gsel[0:3, 0:3 * B], bias_c=enc_bc,
                    ones_row=ones,
                    s0=enc_s0, xc_tile=None, ident=ident, x1t_tile=x1t,
                    c_hist=c_hist, h_hist=None, ifo_ring=ifo_ring,
                    enc_h_chain=enc_h_chain, dense_fn=None,
                    x1t_fetch=x1t_fetch)

        re_t = enc_h_chain[8]  # encoder h_127  [U, B] f16

        # fold dense bias into the accumulators between the phases (db has
        # long since landed; emitting this earlier would head-of-line block
        # the PE behind the db DMA during warmup)
        for ch in range(NCH):
            nc.tensor.matmul(dense_ps[ch // 8][:, (ch % 8) * B:(ch % 8 + 1) * B],
                             db_sb[:, ch * U:(ch + 1) * U], ones[:, :B],
                             start=False, stop=False, skip_group_check=True)

        # ---------------- decoder constant x-term ----------------
        # xc = dec_kernel^T @ RE + dec_bias, gate order [i|f|o|cand]
        xc_big = skip_ps.tile([U, 8 * B], F32, tag="skip_ps",
                              name="xc_big")
        xc_ps = xc_big[:, 0:4 * B]
        for g in range(4):
            gg = REF[g]
            gsl = xc_ps[:, g * B:(g + 1) * B]
            nc.tensor.matmul(gsl, dec_k[:, gg * U:(gg + 1) * U], re_t,
                             start=True, stop=False, skip_group_check=True)
            nc.tensor.matmul(gsl, dec_b[:, gg * U:(gg + 1) * U],
                             ones[:, :B], start=False, stop=True,
                             skip_group_check=True)
        xc = consts.tile([U, 4 * B], F16)
        nc.scalar.copy(xc, xc_ps)

        # ---------------- decoder (+ dense) ----------------
        dec_pend = _lstm_phase(nc, tc, pools, cfg, is_enc=False,
                    k_tiles=dec_k_g, rk_tiles=dec_rk_g, k2_tile=dec_k2,
                    b2_col=dec_b2, bias_ifo=dec_b4[0:3, :],
                    gsel_ifo=gsel[0:3, 0:3 * B], bias_c=dec_bc,
                    ones_row=ones,
                    s0=dec_s0, xc_tile=xc, ident=ident, x1t_tile=None,
                    c_hist=c_hist, h_hist=h_hist, ifo_ring=ifo_ring,
                    enc_h_chain=None, dense_fn=dense_fn, x0_tile=re_t)
        # dense epilogue (last few timesteps' k-tiles)
        for t_ in range(T - 8, T - 1):
            dense_fn(t_)

        # last k-tile: final matmuls, dequant and out-DMA pipelined in
        # [128,256] quarters (bank0 quarters on ACT, bank1 on DVE, DMAs
        # alternating between the sync HWDGE and Pool SWDGE queues)
        out_sb = consts.tile([U, NCH * B], F32)
        ti = (T - 1) // 2
        wt = wf_tiles[ti // GSZ]
        base = (ti % GSZ) * JSH
        lhs_h = h_hist[:, (T - 1 + SKIP) % 32, :]
        Q = 4 * B
        for q in range(4):
            for ch in range(4 * q, 4 * q + 4):
                out = dense_ps[ch // 8][:, (ch % 8) * B:(ch % 8 + 1) * B]
                nc.tensor.matmul(out, wt[:, base + ch * U:base + (ch + 1) * U],
                                 lhs_h, start=False, stop=True,
                                 skip_group_check=True)
            src = dense_ps[q // 2][:, (q % 2) * Q:(q % 2 + 1) * Q]
            dst = out_sb[:, q * Q:(q + 1) * Q]
            if q < 2:
                nc.scalar.activation(dst, src, AF.Identity, scale=1.0 / WSCALE)
            else:
                nc.vector.tensor_scalar_mul(dst, src, 1.0 / WSCALE)
            # four different DGE queues so descriptor generation for all
            # quarters runs in parallel; only the ~360ns transfers serialize
            eng = (nc.sync, nc.gpsimd, nc.scalar, nc.sync)[q]
            eng.dma_start(d_out[:, q * Q:(q + 1) * Q], dst)

    nc.compile()
    return nc


def _prep_inputs(X, enc_kernel, enc_rkernel, enc_kernel2, enc_bias, enc_bias2,
                 dec_kernel, dec_rkernel, dec_kernel2, dec_bias, dec_bias2,
                 dense_w, dense_b):
    """Host-side sharding/layout prep. Returns in_maps (list of 8 dicts)."""
    x1t = np.ascontiguousarray(
        X[:, 0].transpose(2, 1, 0)).astype(nf16)         # (M, T, B)
    common = {
        "x1t": x1t,
        "enc_k": enc_kernel.astype(nf16),
        "enc_rk": enc_rkernel.astype(nf16),
        "enc_k2": enc_kernel2.astype(nf16),
        "dec_k": dec_kernel.astype(nf16),
        "dec_rk": dec_rkernel.astype(nf16),
        "dec_k2": dec_kernel2.astype(nf16),
        # per-gate bias rows in device gate order [i|f|o|cand] for the K=4
        # gate-select start matmul
        "enc_b4": np.stack([enc_bias[r * U:(r + 1) * U]
                            for r in (0, 1, 3, 2)]).astype(nf16),
        "gsel": np.repeat(np.eye(4, dtype=np.float32), B, axis=1).astype(nf16),
        "enc_bc": enc_bias[2 * U:3 * U].reshape(1, -1).astype(nf16),
        "ones": np.ones((1, 512), np.float32).astype(nf16),
        "dec_b": dec_bias.reshape(1, -1).astype(nf16),
        "dec_b4": np.stack([dec_bias[r * U:(r + 1) * U]
                            for r in (0, 1, 3, 2)]).astype(nf16),
        "dec_bc": dec_bias[2 * U:3 * U].reshape(1, -1).astype(nf16),
        "enc_b2": enc_bias2.reshape(-1, 1).astype(np.float32),
        "dec_b2": dec_bias2.reshape(-1, 1).astype(np.float32),
        "ident": np.eye(U, dtype=np.float32).astype(nf16),
    }
    in_maps = []
    for c in range(NCORES):
        m = dict(common)
        # W^T shard [16384 k, 2048 j] -> [tile, p, j] -> [p, tile, j]
        wt = dense_w[c * JSH:(c + 1) * JSH, :].T.reshape(T, U, JSH)
        wt = np.ascontiguousarray(wt.transpose(1, 0, 2)) * WSCALE  # [p,t,j]
        m["we"] = np.ascontiguousarray(wt[:, 0::2, :]).reshape(U, -1).astype(ne3)
        m["wf"] = np.ascontiguousarray(wt[:, 1::2, :]).reshape(U, -1).astype(nf16)
        m["db"] = np.ascontiguousarray(
            dense_b[c * JSH:(c + 1) * JSH].reshape(1, -1) * WSCALE).astype(nf16)
        in_maps.append(m)
    return in_maps


def kernel(X, enc_kernel, enc_rkernel, enc_kernel2, enc_bias, enc_bias2,
           enc_s0, dec_kernel, dec_rkernel, dec_kernel2, dec_bias, dec_bias2,
           dec_s0, dense_w, dense_b, _trace=False):
    cfg = {
        "enc_s0": float(enc_s0),
        "dec_s0": float(dec_s0),
        "we_bufs": 6,
        "wf_bufs": 6,
    }
    key = tuple(sorted(cfg.items()))
    if key not in _GRAPH_CACHE:
        _GRAPH_CACHE[key] = build_graph(cfg)
    nc = _GRAPH_CACHE[key]

    in_maps = _prep_inputs(
        np.asarray(X), np.asarray(enc_kernel), np.asarray(enc_rkernel),
        np.asarray(enc_kernel2), np.asarray(enc_bias), np.asarray(enc_bias2),
        np.asarray(dec_kernel), np.asarray(dec_rkernel), np.asarray(dec_kernel2),
        np.asarray(dec_bias), np.asarray(dec_bias2),
        np.asarray(dense_w), np.asarray(dense_b))

    res = run_bass_kernel_spmd(nc, in_maps, core_ids=list(range(NCORES)),
                               trace=_trace)
    # out[c] is Y^T chunks: [p, ch*64+b] -> Y[b, c*2048 + ch*128 + p]
    parts = []
    for c in range(NCORES):
        buf = res.results[c]["out"].reshape(U, NCH, B)
        parts.append(buf.transpose(2, 1, 0).reshape(B, JSH))
    out = np.concatenate(parts, axis=1).reshape(B, T, M).astype(np.float32)
    if _trace:
        return out, res
    return out


if __name__ == "__main__":
    # smoke test with random data
    rng = np.random.default_rng(0)
    s_in = 1.0 / np.sqrt(M)
    s_u = 1.0 / np.sqrt(U)
    s_d = 1.0 / np.sqrt(T * M)
    inputs = {
        "X": rng.standard_normal((B, 2, T, M), dtype=np.float32),
        "enc_kernel": rng.standard_normal((M, 4 * U), dtype=np.float32) * s_in,
        "enc_rkernel": rng.standard_normal((U, 4 * U), dtype=np.float32) * s_u,
        "enc_kernel2": rng.standard_normal((U, U), dtype=np.float32) * s_u,
        "enc_bias": np.zeros(4 * U, np.float32),
        "enc_bias2": np.zeros(U, np.float32),
        "enc_s0": np.float32(0.5),
        "dec_kernel": rng.standard_normal((U, 4 * U), dtype=np.float32) * s_u,
        "dec_rkernel": rng.standard_normal((U, 4 * U), dtype=np.float32) * s_u,
        "dec_kernel2": rng.standard_normal((U, U), dtype=np.float32) * s_u,
        "dec_bias": np.zeros(4 * U, np.float32),
        "dec_bias2": np.zeros(U, np.float32),
        "dec_s0": np.float32(0.5),
        "dense_w": (rng.standard_normal((T * M, T * M), dtype=np.float32) * s_d),
        "dense_b": np.zeros(T * M, np.float32),
    }
    y = kernel(**inputs)
    print("kernel output", y.shape, y.dtype, float(np.abs(y).mean()))

